# revision 2
# baseline (speedup 1.0000x reference)
"""Trainium2 Bass kernel for the 3-layer LSTM + MLP head (nn_CustomLSTM).

Strategy (pure data parallelism, batch 512 -> 64 per core x 8 cores):

Device layout: everything transposed — [features on partitions, batch on
free dim]. The three LSTM layers run as a wavefront over time (at tick tau,
layer1 computes t=tau, layer2 t=tau-1, layer3 t=tau-2) so the per-step
elementwise work of all three layers packs into single wide instructions
([*, 192] instead of 3x [*, 64]), which is what beats the per-instruction
overhead wall of the 768-step sequential recurrence.

Per tick, one PSUM bank [128, 384] holds all six gate blocks
(cols [L1ig|L2ig|L3ig|L1fo|L2fo|L3fo], rows [i;g] / [f;o]), filled by:
  - one identity matmul adding the (constant) biases,
  - two identity matmuls adding layer1's precomputed input projection xg1,
  - per-layer recurrence matmuls; layers 2/3 use K=128 "cat" weights
    [Whh_l | Wih_l] against rhs [h_l[t-1] ; h_{l-1}[t]].
The g-rows carry 2x-scaled weights so one sigmoid over the whole bank +
(2s-1) gives tanh for g (sigmoid's partition-relocating variant handles the
base-partition constraints).

xg1 = Wih1 @ x^T is precomputed in 512-column chunks on the tensor engine,
streamed ~8 ticks ahead of consumption.
"""
import numpy as np
from contextlib import ExitStack

import concourse.bass as bass
import concourse.tile as tile
from concourse import mybir
from concourse.bass_utils import run_bass_kernel_spmd

F32 = mybir.dt.float32
AF = mybir.ActivationFunctionType
ALU = mybir.AluOpType

B_FULL, T_FULL, D_IN, H, N_CLS = 512, 256, 258, 64, 90
N_CORES = 8
BL = B_FULL // N_CORES  # 64 batch per core

_FC_DIMS = [(64, H), (128, 64), (64, 128), (32, 64), (32, 32), (N_CLS, 32)]


def _split_multiwaits(nc, limit=1):
    """The walrus codegen only supports one semaphore wait per instruction;
    Tile's final drain can carry several. Split extras onto NoOps."""
    for fn in nc.m.functions:
        for bb in fn.blocks:
            out = []
            for inst in bb.instructions:
                si = inst.sync_info
                if si is not None and si.on_wait and len(si.on_wait) > limit:
                    waits = list(si.on_wait)
                    for i in range(0, len(waits) - limit, limit):
                        nop = mybir.InstNoOp(
                            name=nc.get_next_instruction_name(),
                            engine=inst.engine, ins=[], outs=[])
                        nop.sync_info = mybir.SyncInfo(
                            on_wait=waits[i:i + limit], on_update=[])
                        nc.register_instruction(nop)
                        out.append(nop)
                    si.on_wait = waits[len(waits) - limit:]
                out.append(inst)
            bb.instructions[:] = out


def _build(T=T_FULL, loop_n=None, ablate=0):
    assert T % 8 == 0
    NCHUNK = T // 8          # phase-A chunks of 512 cols (8 ticks each)
    NCOLS = T * BL
    nc = bass.Bass()

    xT_d = nc.dram_tensor("xT", [D_IN, NCOLS], F32, kind="ExternalInput")
    wA_d = nc.dram_tensor("wA", [D_IN, 256], F32, kind="ExternalInput")
    id_d = nc.dram_tensor("ident", [128, 128], F32, kind="ExternalInput")
    pb_d = nc.dram_tensor("pbias", [128, 384], F32, kind="ExternalInput")
    w1_d = nc.dram_tensor("w1", [64, 256], F32, kind="ExternalInput")
    c2_d = nc.dram_tensor("cat2", [128, 256], F32, kind="ExternalInput")
    c3_d = nc.dram_tensor("cat3", [128, 256], F32, kind="ExternalInput")
    fcw_d = [nc.dram_tensor(f"fcw{i}", [k, m], F32, kind="ExternalInput")
             for i, (m, k) in enumerate(_FC_DIMS)]
    fcb_d = [nc.dram_tensor(f"fcb{i}", [m, 1], F32, kind="ExternalInput")
             for i, (m, k) in enumerate(_FC_DIMS)]
    out_d = nc.dram_tensor("out", [N_CLS, BL], F32, kind="ExternalOutput")

    with tile.TileContext(nc) as tc, ExitStack() as ctx:
        import contextlib
        const = ctx.enter_context(tc.tile_pool(name="const", bufs=1))
        xgpool = ctx.enter_context(tc.tile_pool(name="xg", bufs=NCHUNK))
        xstage = ctx.enter_context(tc.tile_pool(name="xstage", bufs=3))
        psA = ctx.enter_context(tc.tile_pool(name="psA", bufs=2, space="PSUM"))
        bank = ctx.enter_context(tc.tile_pool(name="bank", bufs=3, space="PSUM"))
        fcps = ctx.enter_context(tc.tile_pool(name="fcps", bufs=1, space="PSUM"))
        work = ctx.enter_context(tc.tile_pool(name="work", bufs=4))
        cats = ctx.enter_context(tc.tile_pool(name="cats", bufs=4))

        dma = nc.sync.dma_start

        # ---- constants -------------------------------------------------
        ident = const.tile([128, 128], F32, tag="ident")
        dma(out=ident, in_=id_d[:])
        pbias = const.tile([128, 384], F32, tag="pbias")
        dma(out=pbias, in_=pb_d[:])
        w1 = const.tile([128, 256], F32, tag="w1")       # data at p64-127
        dma(out=w1[64:128, :], in_=w1_d[:])
        cat2w = const.tile([128, 256], F32, tag="cat2w")
        dma(out=cat2w, in_=c2_d[:])
        cat3w = const.tile([128, 256], F32, tag="cat3w")
        dma(out=cat3w, in_=c3_d[:])
        wA = []
        for blk in range(2):  # 0=ig, 1=fo
            for k0, ksz in ((0, 128), (128, 128), (256, 2)):
                t_ = const.tile([ksz, 128], F32, tag=f"wA{blk}_{k0}")
                dma(out=t_, in_=wA_d[k0:k0 + ksz, blk * 128:(blk + 1) * 128])
                wA.append(t_)
        wA_ig, wA_fo = wA[:3], wA[3:]

        C = const.tile([128, 192], F32, tag="C")         # cell state at p64-127
        nc.vector.memset(C, 0.0)

        loop_cm = tc.For_i(0, loop_n, 1) if loop_n else contextlib.nullcontext()
        Rconst = const.tile([128, 128], F32, tag="Rconst")
        nc.vector.memset(Rconst, 0.0)

        # ---- phase A: xg1 chunks --------------------------------------
        xg_ig = [None] * NCHUNK
        xg_fo = [None] * NCHUNK

        def phase_a(j):
            xa = xstage.tile([128, 512], F32, tag="xa")
            dma(out=xa, in_=xT_d[0:128, j * 512:(j + 1) * 512])
            xb = xstage.tile([128, 512], F32, tag="xb")
            dma(out=xb, in_=xT_d[128:256, j * 512:(j + 1) * 512])
            xc = xstage.tile([2, 512], F32, tag="xc")
            dma(out=xc, in_=xT_d[256:258, j * 512:(j + 1) * 512])
            for blk, (wset, dst) in enumerate(((wA_ig, xg_ig), (wA_fo, xg_fo))):
                p = psA.tile([128, 512], F32, tag=f"psA{blk}")
                nc.tensor.matmul(p, lhsT=wset[0], rhs=xa, start=True, stop=False)
                nc.tensor.matmul(p, lhsT=wset[1], rhs=xb, start=False, stop=False)
                nc.tensor.matmul(p, lhsT=wset[2], rhs=xc, start=False, stop=True)
                g = xgpool.tile([128, 512], F32, tag=f"xg{blk}")
                if blk == 0:
                    nc.scalar.copy(g, p)
                else:
                    nc.vector.tensor_copy(out=g, in_=p)
                dst[j] = g

        with loop_cm:
            phase_a(0)
            if NCHUNK > 1:
                phase_a(1)

            # ---- wavefront over ticks -------------------------------------
            def offchain_mms(tau, P):
                # Bias + xg matmuls for tick tau's bank: independent of the
                # recurrence chain, emitted a tick early so the PE runs them
                # while stalled waiting for R23.
                nc.tensor.matmul(P, lhsT=ident, rhs=pbias, start=True, stop=False,
                                 skip_group_check=True)
                if tau <= T - 1:
                    ch, off = tau // 8, (tau % 8) * 64
                    nc.tensor.matmul(P[:, 0:64], lhsT=ident,
                                     rhs=xg_ig[ch][:, off:off + 64],
                                     start=False, stop=False, skip_group_check=True)
                    nc.tensor.matmul(P[:, 192:256], lhsT=ident,
                                     rhs=xg_fo[ch][:, off:off + 64],
                                     start=False, stop=False, skip_group_check=True)

            R23 = None
            H3 = None
            Pnext = None
            for tau in range(T + 2):
                if tau >= 8 and tau % 8 == 0:
                    j = tau // 8 + 1
                    if j < NCHUNK:
                        phase_a(j)

                l1 = tau <= T - 1
                l2 = 1 <= tau <= T
                l3 = 2 <= tau <= T + 1

                if Pnext is None:
                    P = bank.tile([128, 384], F32, tag="P")
                    offchain_mms(tau, P)
                else:
                    P = Pnext
                if l1 and tau >= 1:
                    nc.tensor.matmul(P[:, 0:64], lhsT=w1[64:128, 0:128],
                                     rhs=R23[64:128, 0:64], start=False, stop=False,
                                     skip_group_check=True)
                    nc.tensor.matmul(P[:, 192:256], lhsT=w1[64:128, 128:256],
                                     rhs=R23[64:128, 0:64], start=False, stop=False,
                                     skip_group_check=True)
                if l2:
                    nc.tensor.matmul(P[:, 64:128], lhsT=cat2w[:, 0:128],
                                     rhs=R23[:, 0:64],
                                     start=False, stop=False, skip_group_check=True)
                    nc.tensor.matmul(P[:, 256:320], lhsT=cat2w[:, 128:256],
                                     rhs=R23[:, 0:64],
                                     start=False, stop=False, skip_group_check=True)
                if l3:
                    nc.tensor.matmul(P[:, 128:192], lhsT=cat3w[:, 0:128],
                                     rhs=R23[:, 64:128],
                                     start=False, stop=False, skip_group_check=True)
                    nc.tensor.matmul(P[:, 320:384], lhsT=cat3w[:, 128:256],
                                     rhs=R23[:, 64:128],
                                     start=False, stop=True, skip_group_check=True)
                if tau + 1 < T + 2:
                    Pnext = bank.tile([128, 384], F32, tag="P")
                    offchain_mms(tau + 1, Pnext)
                else:
                    Pnext = None

                # elementwise, all three layers packed [*, 192]
                # block X rows [i; f], block Y rows [2g; o]:
                #   i = GA[0:64, 0:192]    f = GA[64:128, 0:192]
                #   s = GA[0:64, 192:384]  o = GA[64:128, 192:384]
                GA = work.tile([128, 384], F32, tag="GA")
                if ablate < 3:
                    nc.scalar.activation(GA, P, AF.Sigmoid)
                if ablate >= 2:
                    R23 = Rconst
                    continue
                # U = i*(s-0.5) = i*tanh(ghat)/2, relocated to p64-127
                U = work.tile([128, 192], F32, tag="U")
                nc.vector.scalar_tensor_tensor(
                    out=U[64:128, :], in0=GA[0:64, 192:384], scalar=-0.5,
                    in1=GA[0:64, 0:192], op0=ALU.add, op1=ALU.mult)
                V = work.tile([128, 192], F32, tag="V")
                nc.vector.tensor_tensor(out=V[64:128, :], in0=GA[64:128, 0:192],
                                        in1=C[64:128, :], op=ALU.mult)
                nc.vector.scalar_tensor_tensor(
                    out=C[64:128, :], in0=U[64:128, :], scalar=2.0,
                    in1=V[64:128, :], op0=ALU.mult, op1=ALU.add)
                TC = work.tile([128, 192], F32, tag="TC")    # data at p64-127
                nc.scalar.activation(TC[64:128, :], C[64:128, :], AF.Tanh)

                # h = o * tanh(c) written directly into next tick's rhs tile
                # R23 [128,128]: cols 0-63 = [h2[tau-1]; h1[tau]],
                #                cols 64-127 = [h3[tau-2]; h2[tau-1]]
                R23n = cats.tile([128, 128], F32, tag="R23")
                nc.vector.tensor_tensor(out=R23n[64:128, :],   # h1, h2
                                        in0=GA[64:128, 192:320],
                                        in1=TC[64:128, 0:128], op=ALU.mult)
                nc.vector.tensor_tensor(out=R23n[0:64, :],     # h2, h3 -> base 0
                                        in0=GA[64:128, 256:384],
                                        in1=TC[64:128, 64:192], op=ALU.mult)

                # cell-state resets + zero-h overrides at layer-start ticks
                if tau == 0:
                    nc.vector.memset(C[64:128, 64:128], 0.0)
                    nc.gpsimd.memset(R23n[0:64, 0:64], 0.0)    # h2[-1] = 0
                elif tau == 1:
                    nc.vector.memset(C[64:128, 128:192], 0.0)
                    nc.gpsimd.memset(R23n[0:64, 64:128], 0.0)  # h3[-1] = 0
                R23 = R23n if ablate == 0 else Rconst
                if tau == T + 1:
                    H3 = work.tile([128, 64], F32, tag="H3")   # h3[T-1] at p64-127
                    nc.vector.tensor_tensor(out=H3[64:128, :],
                                            in0=GA[64:128, 320:384],
                                            in1=TC[64:128, 128:192], op=ALU.mult)

            # ---- FC head ---------------------------------------------------
            fcw_s = []
            for i, (m, k) in enumerate(_FC_DIMS):
                if i == 0:  # rhs is h3 at base partition 64
                    t_ = const.tile([128, m], F32, tag=f"fcw{i}")
                    dma(out=t_[64:128, :], in_=fcw_d[i][:])
                    fcw_s.append(t_[64:128, :])
                else:
                    t_ = const.tile([k, m], F32, tag=f"fcw{i}")
                    dma(out=t_, in_=fcw_d[i][:])
                    fcw_s.append(t_)
            fcb_s = []
            for i, (m, k) in enumerate(_FC_DIMS):
                t_ = const.tile([m, 1], F32, tag=f"fcb{i}")
                dma(out=t_, in_=fcb_d[i][:])
                fcb_s.append(t_)

            if ablate >= 2:
                H3 = Rconst
            z = H3[64:128, 0:64]      # h3[T-1], base partition 64
            for i, (m, k) in enumerate(_FC_DIMS):
                pz = fcps.tile([m, 64], F32, tag="fcp")
                nc.tensor.matmul(pz, lhsT=fcw_s[i], rhs=z, start=True, stop=True)
                zs = work.tile([m, 64], F32, tag=f"fz{i}")
                func = AF.Relu if i < 5 else AF.Identity
                nc.scalar.activation(zs, pz, func, bias=fcb_s[i])
                z = zs
            dma(out=out_d[:], in_=z)

    _split_multiwaits(nc)
    return nc


_BUILT = {}


def _get_nc(T=T_FULL, loop_n=None, ablate=0):
    key = (T, loop_n, ablate)
    if key not in _BUILT:
        _BUILT[key] = _build(T, loop_n, ablate)
    return _BUILT[key]


def _sel_ig(W):
    # block X: rows [i; f]
    return np.concatenate([W[0:H], W[H:2 * H]], axis=0)


def _sel_fo(W):
    # block Y: rows [2g; o]
    return np.concatenate([2.0 * W[2 * H:3 * H], W[3 * H:4 * H]], axis=0)


def _prep_weights(inp, T):
    """Host-side weight/bias rearrangement shared by all cores."""
    f32 = np.float32
    Wih1, Whh1 = inp["Wih1"].astype(f32), inp["Whh1"].astype(f32)
    wA = np.concatenate(
        [_sel_ig(Wih1).T, _sel_fo(Wih1).T], axis=1)            # [258, 256]
    w1 = np.concatenate(
        [_sel_ig(Whh1).T, _sel_fo(Whh1).T], axis=1)            # [64, 256]

    def cat_l(l):
        Whh, Wih = inp[f"Whh{l}"].astype(f32), inp[f"Wih{l}"].astype(f32)
        ig = np.concatenate([_sel_ig(Whh).T, _sel_ig(Wih).T], axis=0)
        fo = np.concatenate([_sel_fo(Whh).T, _sel_fo(Wih).T], axis=0)
        return np.concatenate([ig, fo], axis=1)                # [128, 256]

    cat2, cat3 = cat_l(2), cat_l(3)

    pbias = np.zeros((128, 384), f32)
    for l in range(3):
        b = (inp[f"bih{l+1}"] + inp[f"bhh{l+1}"]).astype(f32)
        big = np.concatenate([b[0:H], b[H:2 * H]])
        bfo = np.concatenate([2.0 * b[2 * H:3 * H], b[3 * H:4 * H]])
        pbias[:, l * 64:(l + 1) * 64] = big[:, None]
        pbias[:, 192 + l * 64:192 + (l + 1) * 64] = bfo[:, None]

    m = {
        "wA": np.ascontiguousarray(wA),
        "w1": np.ascontiguousarray(w1),
        "cat2": np.ascontiguousarray(cat2),
        "cat3": np.ascontiguousarray(cat3),
        "pbias": pbias,
        "ident": np.eye(128, dtype=f32),
    }
    for i in range(5):
        m[f"fcw{i}"] = np.ascontiguousarray(inp[f"Wfc{i+1}"].astype(f32).T)
        m[f"fcb{i}"] = np.ascontiguousarray(
            inp[f"bfc{i+1}"].astype(f32)[:, None])
    m["fcw5"] = np.ascontiguousarray(inp["Wout"].astype(f32).T)
    m["fcb5"] = np.ascontiguousarray(inp["bout"].astype(f32)[:, None])
    return m


def run(inputs, trace=False, **rk):
    x = np.asarray(inputs["x"], np.float32)
    B, T, D = x.shape
    nc = _get_nc(T)
    shared = _prep_weights(inputs, T)

    bl = B // N_CORES
    in_maps = []
    for c in range(N_CORES):
        xc = x[c * bl:(c + 1) * bl]                    # [bl, T, D]
        xT = np.ascontiguousarray(xc.transpose(2, 1, 0).reshape(D, T * bl))
        in_maps.append({"xT": xT, **shared})

    bkr = run_bass_kernel_spmd(nc, in_maps, list(range(N_CORES)),
                               trace=trace, **rk)
    res = bkr.results
    out = np.empty((B, N_CLS), np.float32)
    for c in range(N_CORES):
        out[c * bl:(c + 1) * bl] = res[c]["out"].T
    return out, bkr


def kernel(**inputs):
    return run(inputs)[0]



# revision 4
# speedup vs baseline: 1.6478x; 1.6478x over previous
"""Trainium2 Bass kernel for the 3-layer LSTM + MLP head (nn_CustomLSTM).

Strategy (pure data parallelism, batch 512 -> 64 per core x 8 cores):

Device layout: everything transposed — [features on partitions, batch on
free dim]. The three LSTM layers run as a wavefront over time (at tick tau,
layer1 computes t=tau, layer2 t=tau-1, layer3 t=tau-2) so the per-step
elementwise work of all three layers packs into single wide instructions
([*, 192] instead of 3x [*, 64]), which is what beats the per-instruction
overhead wall of the 768-step sequential recurrence.

Per tick, one PSUM bank [128, 384] holds all six gate blocks
(cols [L1ig|L2ig|L3ig|L1fo|L2fo|L3fo], rows [i;g] / [f;o]), filled by:
  - one identity matmul adding the (constant) biases,
  - two identity matmuls adding layer1's precomputed input projection xg1,
  - per-layer recurrence matmuls; layers 2/3 use K=128 "cat" weights
    [Whh_l | Wih_l] against rhs [h_l[t-1] ; h_{l-1}[t]].
The g-rows carry 2x-scaled weights so one sigmoid over the whole bank +
(2s-1) gives tanh for g (sigmoid's partition-relocating variant handles the
base-partition constraints).

xg1 = Wih1 @ x^T is precomputed in 512-column chunks on the tensor engine,
streamed ~8 ticks ahead of consumption.
"""
import numpy as np
import ml_dtypes
from contextlib import ExitStack

BF = ml_dtypes.bfloat16

import concourse.bass as bass
import concourse.tile as tile
from concourse import mybir
from concourse.bass_utils import run_bass_kernel_spmd

F32 = mybir.dt.float32
BF16 = mybir.dt.bfloat16
AF = mybir.ActivationFunctionType
ALU = mybir.AluOpType

B_FULL, T_FULL, D_IN, H, N_CLS = 512, 256, 258, 64, 90
N_CORES = 8
BL = B_FULL // N_CORES  # 64 batch per core

_FC_DIMS = [(64, H), (128, 64), (64, 128), (32, 64), (32, 32), (N_CLS, 32)]


def _split_multiwaits(nc, limit=1):
    """The walrus codegen only supports one semaphore wait per instruction;
    Tile's final drain can carry several. Split extras onto NoOps."""
    for fn in nc.m.functions:
        for bb in fn.blocks:
            out = []
            for inst in bb.instructions:
                si = inst.sync_info
                if si is not None and si.on_wait and len(si.on_wait) > limit:
                    waits = list(si.on_wait)
                    for i in range(0, len(waits) - limit, limit):
                        nop = mybir.InstNoOp(
                            name=nc.get_next_instruction_name(),
                            engine=inst.engine, ins=[], outs=[])
                        nop.sync_info = mybir.SyncInfo(
                            on_wait=waits[i:i + limit], on_update=[])
                        nc.register_instruction(nop)
                        out.append(nop)
                    si.on_wait = waits[len(waits) - limit:]
                out.append(inst)
            bb.instructions[:] = out


def _build(T=T_FULL, loop_n=None, ablate=0):
    assert T % 8 == 0
    NCHUNK = T // 8          # phase-A chunks of 512 cols (8 ticks each)
    NCOLS = T * BL
    nc = bass.Bass()

    xT_d = nc.dram_tensor("xT", [D_IN, NCOLS], BF16, kind="ExternalInput")
    wA_d = nc.dram_tensor("wA", [D_IN, 256], BF16, kind="ExternalInput")
    id_d = nc.dram_tensor("ident", [128, 128], BF16, kind="ExternalInput")
    pb_d = nc.dram_tensor("pbias", [128, 384], BF16, kind="ExternalInput")
    w1_d = nc.dram_tensor("w1", [64, 256], BF16, kind="ExternalInput")
    c2_d = nc.dram_tensor("cat2", [128, 256], BF16, kind="ExternalInput")
    c3_d = nc.dram_tensor("cat3", [128, 256], BF16, kind="ExternalInput")
    fcw_d = [nc.dram_tensor(f"fcw{i}", [k, m], BF16, kind="ExternalInput")
             for i, (m, k) in enumerate(_FC_DIMS)]
    fcb_d = [nc.dram_tensor(f"fcb{i}", [m, 1], F32, kind="ExternalInput")
             for i, (m, k) in enumerate(_FC_DIMS)]
    out_d = nc.dram_tensor("out", [N_CLS, BL], F32, kind="ExternalOutput")

    with tile.TileContext(nc) as tc, ExitStack() as ctx:
        import contextlib
        const = ctx.enter_context(tc.tile_pool(name="const", bufs=1))
        xgpool = ctx.enter_context(tc.tile_pool(name="xg", bufs=NCHUNK))
        xstage = ctx.enter_context(tc.tile_pool(name="xstage", bufs=3))
        psA = ctx.enter_context(tc.tile_pool(name="psA", bufs=2, space="PSUM"))
        bank = ctx.enter_context(tc.tile_pool(name="bank", bufs=3, space="PSUM"))
        fcps = ctx.enter_context(tc.tile_pool(name="fcps", bufs=1, space="PSUM"))
        work = ctx.enter_context(tc.tile_pool(name="work", bufs=4))
        cats = ctx.enter_context(tc.tile_pool(name="cats", bufs=4))

        dma = nc.sync.dma_start

        # ---- constants -------------------------------------------------
        ident = const.tile([128, 128], BF16, tag="ident")
        dma(out=ident, in_=id_d[:])
        pbias = const.tile([128, 384], BF16, tag="pbias")
        dma(out=pbias, in_=pb_d[:])
        w1 = const.tile([128, 256], BF16, tag="w1")       # data at p64-127
        dma(out=w1[64:128, :], in_=w1_d[:])
        cat2w = const.tile([128, 256], BF16, tag="cat2w")
        dma(out=cat2w, in_=c2_d[:])
        cat3w = const.tile([128, 256], BF16, tag="cat3w")
        dma(out=cat3w, in_=c3_d[:])
        wA = []
        for blk in range(2):  # 0=ig, 1=fo
            for k0, ksz in ((0, 128), (128, 128), (256, 2)):
                t_ = const.tile([ksz, 128], BF16, tag=f"wA{blk}_{k0}")
                dma(out=t_, in_=wA_d[k0:k0 + ksz, blk * 128:(blk + 1) * 128])
                wA.append(t_)
        wA_ig, wA_fo = wA[:3], wA[3:]

        C = const.tile([128, 192], BF16, tag="C")         # cell state at p64-127
        nc.vector.memset(C, 0.0)

        loop_cm = tc.For_i(0, loop_n, 1) if loop_n else contextlib.nullcontext()
        Rconst = const.tile([128, 128], BF16, tag="Rconst")
        nc.vector.memset(Rconst, 0.0)

        # ---- phase A: xg1 chunks --------------------------------------
        xg_ig = [None] * NCHUNK
        xg_fo = [None] * NCHUNK

        def phase_a(j):
            xa = xstage.tile([128, 512], BF16, tag="xa")
            dma(out=xa, in_=xT_d[0:128, j * 512:(j + 1) * 512])
            xb = xstage.tile([128, 512], BF16, tag="xb")
            dma(out=xb, in_=xT_d[128:256, j * 512:(j + 1) * 512])
            xc = xstage.tile([2, 512], BF16, tag="xc")
            dma(out=xc, in_=xT_d[256:258, j * 512:(j + 1) * 512])
            for blk, (wset, dst) in enumerate(((wA_ig, xg_ig), (wA_fo, xg_fo))):
                p = psA.tile([128, 512], F32, tag=f"psA{blk}")
                nc.tensor.matmul(p, lhsT=wset[0], rhs=xa, start=True, stop=False)
                nc.tensor.matmul(p, lhsT=wset[1], rhs=xb, start=False, stop=False)
                nc.tensor.matmul(p, lhsT=wset[2], rhs=xc, start=False, stop=True)
                g = xgpool.tile([128, 512], BF16, tag=f"xg{blk}")
                if blk == 0:
                    nc.scalar.copy(g, p)
                else:
                    nc.vector.tensor_copy(out=g, in_=p)
                dst[j] = g

        with loop_cm:
            phase_a(0)
            if NCHUNK > 1:
                phase_a(1)

            # ---- wavefront over ticks -------------------------------------
            def offchain_mms(tau, P):
                # Bias + xg matmuls for tick tau's bank: independent of the
                # recurrence chain, emitted a tick early so the PE runs them
                # while stalled waiting for R23.
                nc.tensor.matmul(P, lhsT=ident, rhs=pbias, start=True, stop=False,
                                 skip_group_check=True)
                if tau <= T - 1:
                    ch, off = tau // 8, (tau % 8) * 64
                    nc.tensor.matmul(P[:, 0:64], lhsT=ident,
                                     rhs=xg_ig[ch][:, off:off + 64],
                                     start=False, stop=False, skip_group_check=True)
                    nc.tensor.matmul(P[:, 192:256], lhsT=ident,
                                     rhs=xg_fo[ch][:, off:off + 64],
                                     start=False, stop=False, skip_group_check=True)

            R23 = None
            H3 = None
            Pnext = None
            for tau in range(T + 2):
                if tau >= 8 and tau % 8 == 0:
                    j = tau // 8 + 1
                    if j < NCHUNK:
                        phase_a(j)

                l1 = tau <= T - 1
                l2 = 1 <= tau <= T
                l3 = 2 <= tau <= T + 1

                if Pnext is None:
                    P = bank.tile([128, 384], F32, tag="P")
                    offchain_mms(tau, P)
                else:
                    P = Pnext
                if l1 and tau >= 1:
                    nc.tensor.matmul(P[:, 0:64], lhsT=w1[64:128, 0:128],
                                     rhs=R23[64:128, 0:64], start=False, stop=False,
                                     skip_group_check=True)
                    nc.tensor.matmul(P[:, 192:256], lhsT=w1[64:128, 128:256],
                                     rhs=R23[64:128, 0:64], start=False, stop=False,
                                     skip_group_check=True)
                if l2:
                    nc.tensor.matmul(P[:, 64:128], lhsT=cat2w[:, 0:128],
                                     rhs=R23[:, 0:64],
                                     start=False, stop=False, skip_group_check=True)
                    nc.tensor.matmul(P[:, 256:320], lhsT=cat2w[:, 128:256],
                                     rhs=R23[:, 0:64],
                                     start=False, stop=False, skip_group_check=True)
                if l3:
                    nc.tensor.matmul(P[:, 128:192], lhsT=cat3w[:, 0:128],
                                     rhs=R23[:, 64:128],
                                     start=False, stop=False, skip_group_check=True)
                    nc.tensor.matmul(P[:, 320:384], lhsT=cat3w[:, 128:256],
                                     rhs=R23[:, 64:128],
                                     start=False, stop=True, skip_group_check=True)
                if tau + 1 < T + 2:
                    Pnext = bank.tile([128, 384], F32, tag="P")
                    offchain_mms(tau + 1, Pnext)
                else:
                    Pnext = None

                # elementwise, all three layers packed [*, 192]
                # block X rows [i; f], block Y rows [2g; o]:
                #   i = GA[0:64, 0:192]    f = GA[64:128, 0:192]
                #   s = GA[0:64, 192:384]  o = GA[64:128, 192:384]
                GA = work.tile([128, 384], BF16, tag="GA")
                if ablate < 3:
                    nc.scalar.activation(GA, P, AF.Sigmoid)
                if ablate >= 2:
                    R23 = Rconst
                    continue
                # U = i*(s-0.5) = i*tanh(ghat)/2, relocated to p64-127
                U = work.tile([128, 192], BF16, tag="U")
                nc.vector.scalar_tensor_tensor(
                    out=U[64:128, :], in0=GA[0:64, 192:384], scalar=-0.5,
                    in1=GA[0:64, 0:192], op0=ALU.add, op1=ALU.mult)
                V = work.tile([128, 192], BF16, tag="V")
                nc.vector.tensor_tensor(out=V[64:128, :], in0=GA[64:128, 0:192],
                                        in1=C[64:128, :], op=ALU.mult)
                nc.vector.scalar_tensor_tensor(
                    out=C[64:128, :], in0=U[64:128, :], scalar=2.0,
                    in1=V[64:128, :], op0=ALU.mult, op1=ALU.add)
                TC = work.tile([128, 192], BF16, tag="TC")    # data at p64-127
                nc.scalar.activation(TC[64:128, :], C[64:128, :], AF.Tanh)

                # h = o * tanh(c) written directly into next tick's rhs tile
                # R23 [128,128]: cols 0-63 = [h2[tau-1]; h1[tau]],
                #                cols 64-127 = [h3[tau-2]; h2[tau-1]]
                R23n = cats.tile([128, 128], BF16, tag="R23")
                nc.vector.tensor_tensor(out=R23n[64:128, :],   # h1, h2
                                        in0=GA[64:128, 192:320],
                                        in1=TC[64:128, 0:128], op=ALU.mult)
                nc.vector.tensor_tensor(out=R23n[0:64, :],     # h2, h3 -> base 0
                                        in0=GA[64:128, 256:384],
                                        in1=TC[64:128, 64:192], op=ALU.mult)

                # cell-state resets + zero-h overrides at layer-start ticks
                if tau == 0:
                    nc.vector.memset(C[64:128, 64:128], 0.0)
                    nc.gpsimd.memset(R23n[0:64, 0:64], 0.0)    # h2[-1] = 0
                elif tau == 1:
                    nc.vector.memset(C[64:128, 128:192], 0.0)
                    nc.gpsimd.memset(R23n[0:64, 64:128], 0.0)  # h3[-1] = 0
                R23 = R23n if ablate == 0 else Rconst
                if tau == T + 1:
                    H3 = work.tile([128, 64], BF16, tag="H3")   # h3[T-1] at p64-127
                    nc.vector.tensor_tensor(out=H3[64:128, :],
                                            in0=GA[64:128, 320:384],
                                            in1=TC[64:128, 128:192], op=ALU.mult)

            # ---- FC head ---------------------------------------------------
            fcw_s = []
            for i, (m, k) in enumerate(_FC_DIMS):
                if i == 0:  # rhs is h3 at base partition 64
                    t_ = const.tile([128, m], BF16, tag=f"fcw{i}")
                    dma(out=t_[64:128, :], in_=fcw_d[i][:])
                    fcw_s.append(t_[64:128, :])
                else:
                    t_ = const.tile([k, m], BF16, tag=f"fcw{i}")
                    dma(out=t_, in_=fcw_d[i][:])
                    fcw_s.append(t_)
            fcb_s = []
            for i, (m, k) in enumerate(_FC_DIMS):
                t_ = const.tile([m, 1], F32, tag=f"fcb{i}")
                dma(out=t_, in_=fcb_d[i][:])
                fcb_s.append(t_)

            if ablate >= 2:
                H3 = Rconst
            z = H3[64:128, 0:64]      # h3[T-1], base partition 64
            for i, (m, k) in enumerate(_FC_DIMS):
                pz = fcps.tile([m, 64], F32, tag="fcp")
                nc.tensor.matmul(pz, lhsT=fcw_s[i], rhs=z, start=True, stop=True)
                zs = work.tile([m, 64], F32 if i == 5 else BF16, tag=f"fz{i}")
                func = AF.Relu if i < 5 else AF.Identity
                nc.scalar.activation(zs, pz, func, bias=fcb_s[i])
                z = zs
            dma(out=out_d[:], in_=z)

    _split_multiwaits(nc)
    return nc


_BUILT = {}


def _get_nc(T=T_FULL, loop_n=None, ablate=0):
    key = (T, loop_n, ablate)
    if key not in _BUILT:
        _BUILT[key] = _build(T, loop_n, ablate)
    return _BUILT[key]


def _sel_ig(W):
    # block X: rows [i; f]
    return np.concatenate([W[0:H], W[H:2 * H]], axis=0)


def _sel_fo(W):
    # block Y: rows [2g; o]
    return np.concatenate([2.0 * W[2 * H:3 * H], W[3 * H:4 * H]], axis=0)


def _prep_weights(inp, T):
    """Host-side weight/bias rearrangement shared by all cores."""
    f32 = np.float32
    Wih1, Whh1 = inp["Wih1"].astype(f32), inp["Whh1"].astype(f32)
    wA = np.concatenate(
        [_sel_ig(Wih1).T, _sel_fo(Wih1).T], axis=1)            # [258, 256]
    w1 = np.concatenate(
        [_sel_ig(Whh1).T, _sel_fo(Whh1).T], axis=1)            # [64, 256]

    def cat_l(l):
        Whh, Wih = inp[f"Whh{l}"].astype(f32), inp[f"Wih{l}"].astype(f32)
        ig = np.concatenate([_sel_ig(Whh).T, _sel_ig(Wih).T], axis=0)
        fo = np.concatenate([_sel_fo(Whh).T, _sel_fo(Wih).T], axis=0)
        return np.concatenate([ig, fo], axis=1)                # [128, 256]

    cat2, cat3 = cat_l(2), cat_l(3)

    pbias = np.zeros((128, 384), f32)
    for l in range(3):
        b = (inp[f"bih{l+1}"] + inp[f"bhh{l+1}"]).astype(f32)
        big = np.concatenate([b[0:H], b[H:2 * H]])
        bfo = np.concatenate([2.0 * b[2 * H:3 * H], b[3 * H:4 * H]])
        pbias[:, l * 64:(l + 1) * 64] = big[:, None]
        pbias[:, 192 + l * 64:192 + (l + 1) * 64] = bfo[:, None]

    m = {
        "wA": np.ascontiguousarray(wA.astype(BF)),
        "w1": np.ascontiguousarray(w1.astype(BF)),
        "cat2": np.ascontiguousarray(cat2.astype(BF)),
        "cat3": np.ascontiguousarray(cat3.astype(BF)),
        "pbias": pbias.astype(BF),
        "ident": np.eye(128, dtype=BF),
    }
    for i in range(5):
        m[f"fcw{i}"] = np.ascontiguousarray(inp[f"Wfc{i+1}"].astype(f32).T.astype(BF))
        m[f"fcb{i}"] = np.ascontiguousarray(
            inp[f"bfc{i+1}"].astype(f32)[:, None])
    m["fcw5"] = np.ascontiguousarray(inp["Wout"].astype(f32).T.astype(BF))
    m["fcb5"] = np.ascontiguousarray(inp["bout"].astype(f32)[:, None])
    return m


def run(inputs, trace=False, **rk):
    x = np.asarray(inputs["x"], np.float32)
    B, T, D = x.shape
    nc = _get_nc(T)
    shared = _prep_weights(inputs, T)

    bl = B // N_CORES
    in_maps = []
    for c in range(N_CORES):
        xc = x[c * bl:(c + 1) * bl]                    # [bl, T, D]
        xT = np.ascontiguousarray(
            xc.transpose(2, 1, 0).reshape(D, T * bl).astype(BF))
        in_maps.append({"xT": xT, **shared})

    bkr = run_bass_kernel_spmd(nc, in_maps, list(range(N_CORES)),
                               trace=trace, **rk)
    res = bkr.results
    out = np.empty((B, N_CLS), np.float32)
    for c in range(N_CORES):
        out[c * bl:(c + 1) * bl] = res[c]["out"].T
    return out, bkr


def kernel(**inputs):
    return run(inputs)[0]



# revision 9
# speedup vs baseline: 1.7735x; 1.0763x over previous
"""Trainium2 Bass kernel for the 3-layer LSTM + MLP head (nn_CustomLSTM).

Strategy (pure data parallelism, batch 512 -> 64 per core x 8 cores):

Device layout: everything transposed — [features on partitions, batch on
free dim]. The three LSTM layers run as a wavefront over time (at tick tau,
layer1 computes t=tau, layer2 t=tau-1, layer3 t=tau-2) so the per-step
elementwise work of all three layers packs into single wide instructions
([*, 192] instead of 3x [*, 64]), which is what beats the per-instruction
overhead wall of the 768-step sequential recurrence.

Per tick, one PSUM bank [128, 384] holds all six gate blocks
(cols [L1ig|L2ig|L3ig|L1fo|L2fo|L3fo], rows [i;g] / [f;o]), filled by:
  - one identity matmul adding the (constant) biases,
  - two identity matmuls adding layer1's precomputed input projection xg1,
  - per-layer recurrence matmuls; layers 2/3 use K=128 "cat" weights
    [Whh_l | Wih_l] against rhs [h_l[t-1] ; h_{l-1}[t]].
The g-rows carry 2x-scaled weights so one sigmoid over the whole bank +
(2s-1) gives tanh for g (sigmoid's partition-relocating variant handles the
base-partition constraints).

xg1 = Wih1 @ x^T is precomputed in 512-column chunks on the tensor engine,
streamed ~8 ticks ahead of consumption.
"""
import numpy as np
import ml_dtypes
from contextlib import ExitStack

BF = ml_dtypes.bfloat16

import concourse.bass as bass
import concourse.tile as tile
from concourse import mybir
from concourse.bass_utils import run_bass_kernel_spmd

F32 = mybir.dt.float32
BF16 = mybir.dt.bfloat16
AF = mybir.ActivationFunctionType
ALU = mybir.AluOpType

B_FULL, T_FULL, D_IN, H, N_CLS = 512, 256, 258, 64, 90
N_CORES = 8
BL = B_FULL // N_CORES  # 64 batch per core

_FC_DIMS = [(64, H), (128, 64), (64, 128), (32, 64), (32, 32), (N_CLS, 32)]


def _split_multiwaits(nc, limit=1):
    """The walrus codegen only supports one semaphore wait per instruction;
    Tile's final drain can carry several. Split extras onto NoOps."""
    for fn in nc.m.functions:
        for bb in fn.blocks:
            out = []
            for inst in bb.instructions:
                si = inst.sync_info
                if si is not None and si.on_wait and len(si.on_wait) > limit:
                    waits = list(si.on_wait)
                    for i in range(0, len(waits) - limit, limit):
                        nop = mybir.InstNoOp(
                            name=nc.get_next_instruction_name(),
                            engine=inst.engine, ins=[], outs=[])
                        nop.sync_info = mybir.SyncInfo(
                            on_wait=waits[i:i + limit], on_update=[])
                        nc.register_instruction(nop)
                        out.append(nop)
                    si.on_wait = waits[len(waits) - limit:]
                out.append(inst)
            bb.instructions[:] = out


def _build(T=T_FULL, loop_n=None, ablate=0):
    assert T % 8 == 0
    NCHUNK = T // 8          # phase-A chunks of 512 cols (8 ticks each)
    NCOLS = T * BL
    nc = bass.Bass()

    xT_d = nc.dram_tensor("xT", [D_IN, NCOLS], BF16, kind="ExternalInput")
    wA_d = nc.dram_tensor("wA", [D_IN, 256], BF16, kind="ExternalInput")
    id_d = nc.dram_tensor("ident", [128, 128], BF16, kind="ExternalInput")
    pb_d = nc.dram_tensor("pbias", [128, 384], BF16, kind="ExternalInput")
    w1_d = nc.dram_tensor("w1", [64, 256], BF16, kind="ExternalInput")
    c2_d = nc.dram_tensor("cat2", [128, 256], BF16, kind="ExternalInput")
    c3_d = nc.dram_tensor("cat3", [128, 256], BF16, kind="ExternalInput")
    fcw_d = [nc.dram_tensor(f"fcw{i}", [k, m], BF16, kind="ExternalInput")
             for i, (m, k) in enumerate(_FC_DIMS)]
    fcb_d = [nc.dram_tensor(f"fcb{i}", [m, 1], F32, kind="ExternalInput")
             for i, (m, k) in enumerate(_FC_DIMS)]
    out_d = nc.dram_tensor("out", [N_CLS, BL], F32, kind="ExternalOutput")

    with tile.TileContext(nc) as tc, ExitStack() as ctx:
        import contextlib
        const = ctx.enter_context(tc.tile_pool(name="const", bufs=1))
        xgpool = ctx.enter_context(tc.tile_pool(name="xg", bufs=NCHUNK))
        xstage = ctx.enter_context(tc.tile_pool(name="xstage", bufs=3))
        psA = ctx.enter_context(tc.tile_pool(name="psA", bufs=1, space="PSUM"))
        bank = ctx.enter_context(tc.tile_pool(name="bank", bufs=2, space="PSUM"))
        fcps = ctx.enter_context(tc.tile_pool(name="fcps", bufs=1, space="PSUM"))
        work = ctx.enter_context(tc.tile_pool(name="work", bufs=8))
        cats = ctx.enter_context(tc.tile_pool(name="cats", bufs=8))

        dma = nc.sync.dma_start

        # ---- constants -------------------------------------------------
        ident = const.tile([128, 128], BF16, tag="ident")
        dma(out=ident, in_=id_d[:])
        pbias = const.tile([128, 384], BF16, tag="pbias")
        dma(out=pbias, in_=pb_d[:])
        w1 = const.tile([128, 256], BF16, tag="w1")       # data at p64-127
        dma(out=w1[64:128, :], in_=w1_d[:])
        cat2w = const.tile([128, 256], BF16, tag="cat2w")
        dma(out=cat2w, in_=c2_d[:])
        cat3w = const.tile([128, 256], BF16, tag="cat3w")
        dma(out=cat3w, in_=c3_d[:])
        wA = []
        for blk in range(2):  # 0=ig, 1=fo
            for k0, ksz in ((0, 128), (128, 128), (256, 2)):
                t_ = const.tile([ksz, 128], BF16, tag=f"wA{blk}_{k0}")
                dma(out=t_, in_=wA_d[k0:k0 + ksz, blk * 128:(blk + 1) * 128])
                wA.append(t_)
        wA_ig, wA_fo = wA[:3], wA[3:]

        D = const.tile([128, 192], BF16, tag="D")         # c/2 state at p64-127
        nc.vector.memset(D, 0.0)

        loop_cm = tc.For_i(0, loop_n, 1) if loop_n else contextlib.nullcontext()
        Rconst = const.tile([128, 128], BF16, tag="Rconst")
        nc.vector.memset(Rconst, 0.0)

        # ---- phase A: xg1 chunks --------------------------------------
        xg_ig = [None] * NCHUNK
        xg_fo = [None] * NCHUNK

        def phase_a(j):
            xa = xstage.tile([128, 512], BF16, tag="xa")
            dma(out=xa, in_=xT_d[0:128, j * 512:(j + 1) * 512])
            xb = xstage.tile([128, 512], BF16, tag="xb")
            dma(out=xb, in_=xT_d[128:256, j * 512:(j + 1) * 512])
            xc = xstage.tile([2, 512], BF16, tag="xc")
            dma(out=xc, in_=xT_d[256:258, j * 512:(j + 1) * 512])
            for blk, (wset, dst) in enumerate(((wA_ig, xg_ig), (wA_fo, xg_fo))):
                p = psA.tile([128, 512], F32, tag=f"psA{blk}")
                nc.tensor.matmul(p, lhsT=wset[0], rhs=xa, start=True, stop=False)
                nc.tensor.matmul(p, lhsT=wset[1], rhs=xb, start=False, stop=False)
                nc.tensor.matmul(p, lhsT=wset[2], rhs=xc, start=False, stop=True)
                g = xgpool.tile([128, 512], BF16, tag=f"xg{blk}")
                if blk == 0:
                    nc.scalar.copy(g, p)
                else:
                    nc.vector.tensor_copy(out=g, in_=p)
                dst[j] = g

        with loop_cm:
            phase_a(0)
            if NCHUNK > 1:
                phase_a(1)

            # ---- wavefront over ticks -------------------------------------
            # Bank is split into two PSUM tiles so sigmoid(X) can fire while
            # the Y-block recurrence matmuls are still streaming:
            #   PX [128,192] cols [L1|L2|L3], rows [i; f]
            #   PY [128,192] cols [L1|L2|L3], rows [2g; o]
            # Cell state kept as D = c/2 so the update is a plain TT add
            # (bf16 2x mode) and tanh(c) = Tanh(D, scale=2) for free.
            def offchain_mms(tau, PX, PY):
                # Bias + xg matmuls for tick tau's banks: independent of the
                # recurrence chain, emitted a tick early so the PE runs them
                # while stalled waiting for R23.
                nc.tensor.matmul(PX, lhsT=ident, rhs=pbias[:, 0:192],
                                 start=True, stop=False, skip_group_check=True)
                nc.tensor.matmul(PY, lhsT=ident, rhs=pbias[:, 192:384],
                                 start=True, stop=False, skip_group_check=True)
                if tau <= T - 1:
                    ch, off = tau // 8, (tau % 8) * 64
                    nc.tensor.matmul(PX[:, 0:64], lhsT=ident,
                                     rhs=xg_ig[ch][:, off:off + 64],
                                     start=False, stop=False, skip_group_check=True)
                    nc.tensor.matmul(PY[:, 0:64], lhsT=ident,
                                     rhs=xg_fo[ch][:, off:off + 64],
                                     start=False, stop=False, skip_group_check=True)

            R23 = None
            H3 = None
            Pnext = None
            for tau in range(T + 2):
                if tau >= 8 and tau % 8 == 0:
                    j = tau // 8 + 1
                    if j < NCHUNK:
                        phase_a(j)

                l1 = tau <= T - 1
                l2 = 1 <= tau <= T
                l3 = 2 <= tau <= T + 1

                if Pnext is None:
                    PX = bank.tile([128, 192], F32, tag="PX")
                    PY = bank.tile([128, 192], F32, tag="PY")
                    offchain_mms(tau, PX, PY)
                else:
                    PX, PY = Pnext
                # X block first (L1 first: it only needs TT1's h1), so
                # sigmoid(X) can issue while the Y matmuls stream.
                if l1 and tau >= 1:
                    nc.tensor.matmul(PX[:, 0:64], lhsT=w1[64:128, 0:128],
                                     rhs=R23[64:128, 0:64], start=False, stop=False,
                                     skip_group_check=True)
                if l2:
                    nc.tensor.matmul(PX[:, 64:128], lhsT=cat2w[:, 0:128],
                                     rhs=R23[:, 0:64],
                                     start=False, stop=False, skip_group_check=True)
                if l3:
                    nc.tensor.matmul(PX[:, 128:192], lhsT=cat3w[:, 0:128],
                                     rhs=R23[:, 64:128],
                                     start=False, stop=not l1, skip_group_check=True)
                if l1 and tau >= 1:
                    nc.tensor.matmul(PY[:, 0:64], lhsT=w1[64:128, 128:256],
                                     rhs=R23[64:128, 0:64], start=False, stop=False,
                                     skip_group_check=True)
                if l2:
                    nc.tensor.matmul(PY[:, 64:128], lhsT=cat2w[:, 128:256],
                                     rhs=R23[:, 0:64],
                                     start=False, stop=False, skip_group_check=True)
                if l3:
                    nc.tensor.matmul(PY[:, 128:192], lhsT=cat3w[:, 128:256],
                                     rhs=R23[:, 64:128],
                                     start=False, stop=True, skip_group_check=True)
                if tau + 1 < T + 2:
                    Pnext = (bank.tile([128, 192], F32, tag="PX", name="PXn"),
                             bank.tile([128, 192], F32, tag="PY", name="PYn"))
                    offchain_mms(tau + 1, *Pnext)
                else:
                    Pnext = None

                # elementwise, all three layers packed [*, 192]
                #   i = GAX[0:64, :]   f = GAX[64:128, :]
                #   s = GAY[0:64, :]   o = GAY[64:128, :]
                GAX = work.tile([128, 192], BF16, tag="GAX")
                GAY = work.tile([128, 192], BF16, tag="GAY")
                if ablate < 3:
                    nc.scalar.activation(GAX, PX, AF.Sigmoid)
                    nc.scalar.activation(GAY, PY, AF.Sigmoid)
                if ablate >= 2:
                    R23 = Rconst
                    continue
                # V = f * d  (needs only sigmoid(X): overlaps sigmoid(Y))
                V = work.tile([128, 192], BF16, tag="V")
                nc.vector.tensor_tensor(out=V[64:128, :], in0=GAX[64:128, :],
                                        in1=D[64:128, :], op=ALU.mult)
                # U = i*(s-0.5) = i*tanh(ghat)/2, relocated to p64-127
                U = work.tile([128, 192], BF16, tag="U")
                nc.vector.scalar_tensor_tensor(
                    out=U[64:128, :], in0=GAY[0:64, :], scalar=-0.5,
                    in1=GAX[0:64, :], op0=ALU.add, op1=ALU.mult)
                # d' = f*d + i*(s-0.5)  (= c/2)
                nc.vector.tensor_tensor(out=D[64:128, :], in0=U[64:128, :],
                                        in1=V[64:128, :], op=ALU.add)
                TC = work.tile([128, 192], BF16, tag="TC")    # data at p64-127
                nc.scalar.activation(TC[64:128, :], D[64:128, :], AF.Tanh,
                                     scale=2.0)

                # h = o * tanh(c) written directly into next tick's rhs tile
                # R23 [128,128]: cols 0-63 = [h2[tau-1]; h1[tau]],
                #                cols 64-127 = [h3[tau-2]; h2[tau-1]]
                R23n = cats.tile([128, 128], BF16, tag="R23")
                nc.vector.tensor_tensor(out=R23n[64:128, :],   # h1, h2
                                        in0=GAY[64:128, 0:128],
                                        in1=TC[64:128, 0:128], op=ALU.mult)
                nc.vector.tensor_tensor(out=R23n[0:64, :],     # h2, h3 -> base 0
                                        in0=GAY[64:128, 64:192],
                                        in1=TC[64:128, 64:192], op=ALU.mult)

                # cell-state resets + zero-h overrides at layer-start ticks
                if tau == 0:
                    nc.vector.memset(D[64:128, 64:128], 0.0)
                    nc.gpsimd.memset(R23n[0:64, 0:64], 0.0)    # h2[-1] = 0
                elif tau == 1:
                    nc.vector.memset(D[64:128, 128:192], 0.0)
                    nc.gpsimd.memset(R23n[0:64, 64:128], 0.0)  # h3[-1] = 0
                R23 = R23n if ablate == 0 else Rconst
                if tau == T + 1:
                    H3 = work.tile([128, 64], BF16, tag="H3")   # h3[T-1] at p64-127
                    nc.vector.tensor_tensor(out=H3[64:128, :],
                                            in0=GAY[64:128, 128:192],
                                            in1=TC[64:128, 128:192], op=ALU.mult)

            # ---- FC head ---------------------------------------------------
            fcw_s = []
            for i, (m, k) in enumerate(_FC_DIMS):
                if i == 0:  # rhs is h3 at base partition 64
                    t_ = const.tile([128, m], BF16, tag=f"fcw{i}")
                    dma(out=t_[64:128, :], in_=fcw_d[i][:])
                    fcw_s.append(t_[64:128, :])
                else:
                    t_ = const.tile([k, m], BF16, tag=f"fcw{i}")
                    dma(out=t_, in_=fcw_d[i][:])
                    fcw_s.append(t_)
            fcb_s = []
            for i, (m, k) in enumerate(_FC_DIMS):
                t_ = const.tile([m, 1], F32, tag=f"fcb{i}")
                dma(out=t_, in_=fcb_d[i][:])
                fcb_s.append(t_)

            if ablate >= 2:
                H3 = Rconst
            z = H3[64:128, 0:64]      # h3[T-1], base partition 64
            for i, (m, k) in enumerate(_FC_DIMS):
                pz = fcps.tile([m, 64], F32, tag="fcp")
                nc.tensor.matmul(pz, lhsT=fcw_s[i], rhs=z, start=True, stop=True)
                zs = work.tile([m, 64], F32 if i == 5 else BF16, tag=f"fz{i}")
                func = AF.Relu if i < 5 else AF.Identity
                nc.scalar.activation(zs, pz, func, bias=fcb_s[i])
                z = zs
            dma(out=out_d[:], in_=z)

    _split_multiwaits(nc)
    return nc


_BUILT = {}


def _get_nc(T=T_FULL, loop_n=None, ablate=0):
    key = (T, loop_n, ablate)
    if key not in _BUILT:
        _BUILT[key] = _build(T, loop_n, ablate)
    return _BUILT[key]


def _sel_ig(W):
    # block X: rows [i; f]
    return np.concatenate([W[0:H], W[H:2 * H]], axis=0)


def _sel_fo(W):
    # block Y: rows [2g; o]
    return np.concatenate([2.0 * W[2 * H:3 * H], W[3 * H:4 * H]], axis=0)


def _prep_weights(inp, T):
    """Host-side weight/bias rearrangement shared by all cores."""
    f32 = np.float32
    Wih1, Whh1 = inp["Wih1"].astype(f32), inp["Whh1"].astype(f32)
    wA = np.concatenate(
        [_sel_ig(Wih1).T, _sel_fo(Wih1).T], axis=1)            # [258, 256]
    w1 = np.concatenate(
        [_sel_ig(Whh1).T, _sel_fo(Whh1).T], axis=1)            # [64, 256]

    def cat_l(l):
        Whh, Wih = inp[f"Whh{l}"].astype(f32), inp[f"Wih{l}"].astype(f32)
        ig = np.concatenate([_sel_ig(Whh).T, _sel_ig(Wih).T], axis=0)
        fo = np.concatenate([_sel_fo(Whh).T, _sel_fo(Wih).T], axis=0)
        return np.concatenate([ig, fo], axis=1)                # [128, 256]

    cat2, cat3 = cat_l(2), cat_l(3)

    pbias = np.zeros((128, 384), f32)
    for l in range(3):
        b = (inp[f"bih{l+1}"] + inp[f"bhh{l+1}"]).astype(f32)
        big = np.concatenate([b[0:H], b[H:2 * H]])
        bfo = np.concatenate([2.0 * b[2 * H:3 * H], b[3 * H:4 * H]])
        pbias[:, l * 64:(l + 1) * 64] = big[:, None]
        pbias[:, 192 + l * 64:192 + (l + 1) * 64] = bfo[:, None]

    m = {
        "wA": np.ascontiguousarray(wA.astype(BF)),
        "w1": np.ascontiguousarray(w1.astype(BF)),
        "cat2": np.ascontiguousarray(cat2.astype(BF)),
        "cat3": np.ascontiguousarray(cat3.astype(BF)),
        "pbias": pbias.astype(BF),
        "ident": np.eye(128, dtype=BF),
    }
    for i in range(5):
        m[f"fcw{i}"] = np.ascontiguousarray(inp[f"Wfc{i+1}"].astype(f32).T.astype(BF))
        m[f"fcb{i}"] = np.ascontiguousarray(
            inp[f"bfc{i+1}"].astype(f32)[:, None])
    m["fcw5"] = np.ascontiguousarray(inp["Wout"].astype(f32).T.astype(BF))
    m["fcb5"] = np.ascontiguousarray(inp["bout"].astype(f32)[:, None])
    return m


def run(inputs, trace=False, **rk):
    x = np.asarray(inputs["x"], np.float32)
    B, T, D = x.shape
    nc = _get_nc(T)
    shared = _prep_weights(inputs, T)

    bl = B // N_CORES
    in_maps = []
    for c in range(N_CORES):
        xc = x[c * bl:(c + 1) * bl]                    # [bl, T, D]
        xT = np.ascontiguousarray(
            xc.transpose(2, 1, 0).reshape(D, T * bl).astype(BF))
        in_maps.append({"xT": xT, **shared})

    bkr = run_bass_kernel_spmd(nc, in_maps, list(range(N_CORES)),
                               trace=trace, **rk)
    res = bkr.results
    out = np.empty((B, N_CLS), np.float32)
    for c in range(N_CORES):
        out[c * bl:(c + 1) * bl] = res[c]["out"].T
    return out, bkr


def kernel(**inputs):
    return run(inputs)[0]

